# revision 9
# baseline (speedup 1.0000x reference)
"""Ragged-segment attention for Trainium2 (8 NeuronCores, SPMD), bin-dense fp16.

Per-segment masking/softmax structure is folded into a host-built low-rank
additive mask applied with ONE matmul per bin:
    mask[q,k] = (kb[k] + NEG) * 1  +  sum_s (-NEG) * 1_s[q] 1_s[k]
so scores/softmax/exp-transpose/out are all dense [128 x 128] bin ops and
segments pack at arbitrary offsets (first-fit decreasing, ~97% dense bins).

The context is shipped in BOTH layouts from the host (natural token-major +
d-major transposed), partition-major so each group is ONE big-descriptor DMA.
This removes all context transposes (PE) and their PSUM->SBUF copies (DVE)
from the device, leaving PE dominated by the irreducible u = tanh(C W^T)
projection.
"""
import numpy as np

import concourse.bacc as bacc
import concourse.mybir as mybir
import concourse.tile as tile
from concourse.bass_utils import run_bass_kernel_spmd

F32 = mybir.dt.float32
FP16 = mybir.dt.float16

N_CORES = 8
D = 512
BIN = 128
GROUP = 8

LAST_RESULTS = {}


def _plan(lengths, mode="fp16"):
    S = len(lengths)
    n_slots = S // N_CORES
    order = np.argsort(-lengths, kind="stable")
    seg_ids = [[int(order[N_CORES * j + c]) for j in range(n_slots)]
               for c in range(N_CORES)]
    slot_len = [int(lengths[order[N_CORES * j]]) for j in range(n_slots)]

    bins = []   # (used-token count, n_segs) per bin
    slots = []  # (bin, off, L)
    for j, L in enumerate(slot_len):
        bi = next((i for i, (used, ns) in enumerate(bins)
                   if used + L <= BIN and ns < 31), None)
        if bi is None:
            bins.append((0, 0))
            bi = len(bins) - 1
        used, ns = bins[bi]
        slots.append((bi, used, L))
        bins[bi] = (used + L, ns + 1)
    n_bins = len(bins)
    return slots, n_bins, seg_ids


def _group_sizes(n_bins):
    """Split n_bins into groups of GROUP with one smaller tail group."""
    sizes = [GROUP] * (n_bins // GROUP)
    if n_bins % GROUP:
        sizes.append(n_bins % GROUP)
    return sizes


def _mask_layout(slots, n_bins):
    by_bin = [[] for _ in range(n_bins)]
    for bi, off, L in slots:
        by_bin[bi].append((off, L))
    kmask = [len(by_bin[b]) + 1 for b in range(n_bins)]
    assert max(kmask) <= 32
    return by_bin, kmask


def _build(slots, n_bins, mode="fp16", repeat=1):
    nc = bacc.Bacc("TRN2", target_bir_lowering=False)
    sizes = _group_sizes(n_bins)
    n_groups = len(sizes)
    g_first = [sum(sizes[:i]) for i in range(n_groups)]

    by_bin, kmask = _mask_layout(slots, n_bins)

    # cb: per bin 1024 cols: [:512] natural (partition=token), [512:]
    # transposed by 128-chunk (partition=d_local, cols (k,chunk-token)).
    cb_d = nc.dram_tensor("cb", [128, n_bins * 1024], FP16, kind="ExternalInput")
    wt = nc.dram_tensor("wt", [128, 4 * D], FP16, kind="ExternalInput")
    bvec = nc.dram_tensor("bvec", [128, 4], F32, kind="ExternalInput")
    # per-group masks: [128, half, type, 128]; bin i at band 32*(i%4),
    # half i//4; type 0 = q-indicator rows, type 1 = kb/-NEG rows.
    msk = nc.dram_tensor("msk", [n_groups * 128, 512], FP16,
                         kind="ExternalInput")
    opk = nc.dram_tensor("opk", [128, n_bins * D], FP16, kind="ExternalOutput")

    ident = nc.inline_tensor(np.eye(128, dtype=np.float16), name="ident")

    with tile.TileContext(nc) as tc:
        with (
            tc.tile_pool(name="const", bufs=1) as cpool,
            tc.tile_pool(name="cb", bufs=3) as cbp,
            tc.tile_pool(name="ut", bufs=2) as utp,
            tc.tile_pool(name="seg", bufs=4) as segp,
            tc.tile_pool(name="stat", bufs=8) as statp,
            tc.tile_pool(name="outp", bufs=2) as outp,
            tc.tile_pool(name="mk", bufs=3) as mkp,
            tc.tile_pool(name="ups", bufs=2, space="PSUM") as ups,
            tc.tile_pool(name="scps", bufs=2, space="PSUM") as scps,
            tc.tile_pool(name="teps", bufs=2, space="PSUM") as teps,
            tc.tile_pool(name="ops", bufs=2, space="PSUM") as opsp,
        ):
            wt_sb = cpool.tile([128, 4, D], FP16, tag="wt")
            b_sb = cpool.tile([128, 4], F32, tag="b")
            id_t = cpool.tile([128, 128], FP16, tag="id")
            nc.sync.dma_start(wt_sb[:], wt.ap().rearrange("p (c e) -> p c e", c=4))
            nc.sync.dma_start(b_sb[:], bvec[:])
            nc.sync.dma_start(id_t[:], ident[:])

            cb_v = cb_d.ap().rearrange("p (b e) -> p b e", e=1024)
            opk_v = opk.ap().rearrange("p (b e) -> p b e", e=D)
            msk_v = msk.ap().rearrange("(g r) (h t p) -> g r h t p", t=2, p=128,
                                       g=n_groups)

            def load_group(g):
                gs = sizes[g]
                b0 = g_first[g]
                cg = cbp.tile([128, gs, 1024], FP16, tag="cg")
                nc.sync.dma_start(cg[:], cb_v[:, b0:b0 + gs, :])
                mg = mkp.tile([128, 2, 2, 128], FP16, tag="mg")
                nc.sync.dma_start(mg[:], msk_v[g])
                return cg, mg

            def u_chunk(st, c):
                """u^T rows for d_out chunk c, all bins of the group."""
                cg, ut, gs = st["cg"], st["ut"], st["gs"]
                nh = (gs + 3) // 4
                for h in range(nh):
                    hb = min(4, gs - 4 * h)
                    ups_t = ups.tile([128, 512], F32, tag="ups")
                    for k in range(4):
                        nc.tensor.matmul(
                            ups_t[:, :hb * 128],
                            wt_sb[:, k, c * 128:(c + 1) * 128],
                            cg[:, 4 * h:4 * h + hb, 512 + k * 128:512 + (k + 1) * 128],
                            start=(k == 0), stop=(k == 3))
                    nc.scalar.activation(
                        ut[:, c, 4 * h:4 * h + hb, :], ups_t[:, :hb * 128],
                        mybir.ActivationFunctionType.Tanh, bias=b_sb[:, c:c + 1])

            def bin_scores(st, i):
                g = st["g"]
                b = g_first[g] + i
                if not by_bin[b]:
                    return
                cg, ut, mg = st["cg"], st["ut"], st["mg"]
                km = kmask[b]
                band, half = 32 * (i % 4), i // 4
                sc = scps.tile([128, 128], F32, tag="sc")
                for k in range(4):
                    nc.tensor.matmul(
                        sc[:], cg[:, i, 512 + k * 128:512 + (k + 1) * 128],
                        ut[:, k, i, :], start=(k == 0), stop=False)
                nc.tensor.matmul(sc[:], mg[band:band + km, half, 0, :],
                                 mg[band:band + km, half, 1, :],
                                 start=False, stop=True,
                                 tile_position=(band, 0))

                nmax = statp.tile([128, 1], F32, tag="nmax")
                sums = statp.tile([128, 1], F32, tag="sums")
                recip = statp.tile([128, 1], F32, tag="recip")
                expt = segp.tile([128, 128], FP16, tag="expt")
                nc.vector.tensor_reduce(
                    nmax[:], sc[:], axis=mybir.AxisListType.X,
                    op=mybir.AluOpType.max, negate=True)
                nc.scalar.activation(
                    expt[:], sc[:], mybir.ActivationFunctionType.Exp,
                    bias=nmax[:], accum_out=sums[:])
                nc.vector.reciprocal(recip[:], sums[:])
                st[("bin", i)] = (expt, recip)

            def bin_out(st, i, use_act_copy):
                if ("bin", i) not in st:
                    return
                expt, recip = st.pop(("bin", i))
                cg, og = st["cg"], st["og"]
                tp = teps.tile([128, 128], FP16, tag="te")
                nc.tensor.transpose(tp[:], expt[:], id_t[:])
                attn = segp.tile([128, 128], FP16, tag="attn")
                nc.vector.tensor_copy(attn[:], tp[:])

                ops_t = opsp.tile([128, D], F32, tag="ops")
                nc.tensor.matmul(ops_t[:], attn[:], cg[:, i, :512],
                                 start=True, stop=True)
                # normalize rows by 1/sum during the psum->sbuf copy
                if use_act_copy:
                    nc.scalar.activation(og[:, i, :], ops_t[:],
                                         mybir.ActivationFunctionType.Copy,
                                         scale=recip[:])
                else:
                    nc.vector.tensor_scalar_mul(og[:, i, :], ops_t[:], recip[:])

            def store_group(st):
                g = st["g"]
                b0 = g_first[g]
                # SP HWDGE queue: loads are prefetched 2 iterations ahead, so
                # a store head-blocking the SP queue costs nothing, while on
                # the ACT queue it would delay the next group's exp chain.
                nc.sync.dma_start(
                    opk_v[:, b0:b0 + st["gs"], :], st["og"])

            # software pipeline over groups:
            #   iter it: attention(bins of group g=it) using ut computed at
            #   it-1 and cg loaded at it-2; u-matmuls+tanh for group g+1
            #   (cg loaded at it-1) interleaved between bins; prefetch DMA
            #   for group g+2.
            niter = repeat * n_groups
            states = {}
            for it in range(niter + 2):
                # prefetch group it (used for u at it+1, attention at it+2)
                if it < niter:
                    g = it % n_groups
                    cg, mg = load_group(g)
                    states[it] = {"g": g, "gs": sizes[g], "cg": cg, "mg": mg}
                st_u = states.get(it - 1)     # compute u for this group
                st_a = states.pop(it - 2, None)  # run attention for this one

                if st_u is not None:
                    ut_t = utp.tile([128, 4, st_u["gs"], 128], FP16, tag="ut")
                    og_t = outp.tile([128, st_u["gs"], D], FP16, tag="og")
                    st_u["ut"] = ut_t
                    st_u["og"] = og_t
                if st_a is None:
                    if st_u is not None:
                        for c in range(4):
                            u_chunk(st_u, c)
                    continue

                gs = st_a["gs"]
                pend = []
                ci = 0
                for i in range(gs):
                    bin_scores(st_a, i)
                    pend.append(i)
                    if len(pend) > 3:
                        j = pend.pop(0)
                        bin_out(st_a, j, use_act_copy=False)
                    if st_u is not None and i % 2 == 1 and ci < 4:
                        u_chunk(st_u, ci)
                        ci += 1
                if st_u is not None:
                    while ci < 4:
                        u_chunk(st_u, ci)
                        ci += 1
                for j in pend:
                    bin_out(st_a, j, use_act_copy=False)
                store_group(st_a)

    nc.compile()
    return nc


def _host_arrays(slots, n_bins, seg_ids, lengths, context, W, b, mode="fp16"):
    NEG = -30000.0
    sizes = _group_sizes(n_bins)
    n_groups = len(sizes)
    g_first = [sum(sizes[:i]) for i in range(n_groups)]
    by_bin2 = [[] for _ in range(n_bins)]
    for j, (bi, off, L) in enumerate(slots):
        by_bin2[bi].append((j, off, L))

    wt = np.ascontiguousarray(
        W.T.reshape(4, 128, D).transpose(1, 0, 2).reshape(128, 4 * D)
    ).astype(np.float16)
    bvec = np.ascontiguousarray(b.reshape(4, 128).T).astype(np.float32)

    in_maps = []
    for c in range(N_CORES):
        cpk = np.zeros((n_bins, BIN, D), np.float16)
        kb = np.full((n_bins, BIN), NEG, np.float32)
        for j, (bi, off, _L) in enumerate(slots):
            s = seg_ids[c][j]
            n = int(lengths[s])
            cpk[bi, off:off + n] = context[s, :n].astype(np.float16)
            kb[bi, off:off + n] = 0.0
        # cb[p, b, :512] natural (p=token); cb[p, b, 512+128k+t] = cpk[b,t,128k+p]
        cb = np.empty((128, n_bins, 1024), np.float16)
        cb[:, :, :512] = cpk.transpose(1, 0, 2)
        cb[:, :, 512:] = (cpk.reshape(n_bins, 128, 4, 128)
                          .transpose(3, 0, 2, 1).reshape(128, n_bins, 512))
        msk = np.zeros((n_groups, 128, 2, 2, 128), np.float32)
        for bb in range(n_bins):
            g = 0
            while not (g_first[g] <= bb < g_first[g] + sizes[g]):
                g += 1
            i = bb - g_first[g]
            band, half = 32 * (i % 4), i // 4
            msk[g, band, half, 0] = 1.0
            msk[g, band, half, 1] = kb[bb] + NEG
            for r, (_j, off, L) in enumerate(by_bin2[bb]):
                msk[g, band + 1 + r, half, 0, off:off + L] = 1.0
                msk[g, band + 1 + r, half, 1, off:off + L] = -NEG
        in_maps.append({
            "cb": np.ascontiguousarray(cb.reshape(128, n_bins * 1024)),
            "wt": wt, "bvec": bvec,
            "msk": msk.reshape(n_groups * 128, 512).astype(np.float16),
        })
    return in_maps


_CACHE = {}


def kernel(context, lengths, W, b, mode="fp16"):
    context = np.asarray(context, dtype=np.float32)
    lengths = np.asarray(lengths, dtype=np.int32)
    W = np.asarray(W, dtype=np.float32)
    b = np.asarray(b, dtype=np.float32)
    S, Lmax, Din = context.shape

    slots, n_bins, seg_ids = _plan(lengths, mode)
    key = (tuple(slots), n_bins, mode)
    if key in _CACHE:
        nc = _CACHE[key]
    else:
        nc = _build(slots, n_bins, mode)
        _CACHE[key] = nc

    in_maps = _host_arrays(slots, n_bins, seg_ids, lengths, context, W, b, mode)
    res = run_bass_kernel_spmd(nc, in_maps, list(range(N_CORES)))
    LAST_RESULTS["exec_time_ns"] = res.exec_time_ns

    out = np.zeros((S, Lmax, D), np.float32)
    for c in range(N_CORES):
        opk = res.results[c]["opk"].astype(np.float32).reshape(128, n_bins, D)
        for j, (bi, off, _L) in enumerate(slots):
            s = seg_ids[c][j]
            n = int(lengths[s])
            out[s, :n] = opk[off:off + n, bi]
    return out
